# revision 10
# baseline (speedup 1.0000x reference)
"""Class-based decoder (MoE-style routing) on 8 trn2 NeuronCores.

Strategy: expert-parallel. Classes are padded 250->256 and split 32 per core.
On the host, tokens are grouped by class into capacity-padded slots (C tokens
per class slot); the rare tokens beyond a class's capacity are evaluated on
the host.  Each core receives its padded tokens pre-transposed k-major (xT,
bf16) plus the replicated class-decoder weights and its word-decoder shard,
both quantized to float8_e3m4 pre-scaled by 64 (exact power of two, divided
back out on the host).  Per 128-token m-tile the PE computes class logits
(x @ Wc.T) and, per pair of classes, word logits (x @ [W_a|W_b].T) as K=512
accumulations with x as the stationary operand and the weights as the moving
operand, so every weight element streams through the PE exactly once.  Each
2C-row band's full pair block is copied out bf16 (one DVE cast per band) and
the host picks the diagonal during the unpermute — on-device selection costs
~420ns of fixed DVE overhead per extra instruction, which was the previous
bottleneck.

Why fp8: the 6.5 MB/core weight stream is the memory-bound bottleneck; e3m4
(4 mantissa bits) at scale 64 keeps all values normal and contributes ~1.2%
relative error (measured offline), under the 2e-2 gate, while x (stationary,
shared by both matmuls) stays bf16.
"""

import numpy as np
from contextlib import ExitStack

import concourse.bass as bass
import concourse.bacc as bacc
import concourse.tile as tile
import concourse.mybir as mybir
from concourse.bass_utils import run_bass_kernel_spmd

NHID = 512
NCLS = 250
CHUNK = 200
NCORES = 8
KCH = NHID // 128          # 4 contraction chunks of 128
NCLS_PAD = 256             # classes padded so each core owns an equal shard
CPC = NCLS_PAD // NCORES   # classes per core
NCOL = NCLS + CHUNK        # 450 output columns
F32 = mybir.dt.float32
BF16 = mybir.dt.bfloat16
F8 = mybir.dt.float8e3    # e3m4
WSCALE = 64.0              # weight pre-scale (power of two; divided out on host)
NWARM = 32                 # PE warm-up matmuls (HAM unthrottle + DMA ramp)

LAST_RESULT = None         # BassKernelResults of the most recent device run
_program_cache = {}

try:
    import ml_dtypes
    _BF16_NP = ml_dtypes.bfloat16
    _F8_NP = ml_dtypes.float8_e3m4
except ImportError:  # pragma: no cover - ml_dtypes ships with jax
    _BF16_NP = None
    _F8_NP = None


def _build_program(C, slots):
    """One SPMD program: slots class-slots of C tokens each, per core."""
    n_mt = (slots * C) // 128  # 128-token m-tiles
    npad = slots * C
    per_mt = 128 // C          # class slots per m-tile
    gs = 2 if per_mt >= 2 else 1
    gw = gs * CHUNK            # moving-operand width per word matmul
    n_half = per_mt // gs      # word matmul groups per m-tile
    hw = KCH * gw              # free-dim elems per half
    ncls_p = 256
    ocol = NCLS + gw           # wide rows: full pair block, host picks diag

    def wchunks(m):
        # W DMA chunks per m-tile as (first_half, n_halves): fine-grained on
        # the first m-tile (compute starts sooner) and the last (short tail)
        if n_half == 1 or m == 0 or m == n_mt - 1:
            return [(h, 1) for h in range(n_half)]
        return [(2 * q, 2) for q in range(n_half // 2)]

    nc = bacc.Bacc("TRN2", target_bir_lowering=False, debug=False,
                   num_devices=NCORES)
    xT = nc.dram_tensor("xT", [128, n_mt * KCH * 128], BF16,
                        kind="ExternalInput")
    wcT = nc.dram_tensor("wcT", [128, KCH * ncls_p], F8,
                         kind="ExternalInput")
    wwT = nc.dram_tensor("wwT", [n_mt, 128, n_half * hw], F8,
                         kind="ExternalInput")
    out = nc.dram_tensor("out", [npad, ocol], BF16, kind="ExternalOutput")

    with tile.TileContext(nc) as tc, ExitStack() as ctx:
        xpool = ctx.enter_context(tc.tile_pool(name="x", bufs=n_mt))
        wcpool = ctx.enter_context(tc.tile_pool(name="wc", bufs=1))
        wpool = ctx.enter_context(tc.tile_pool(name="w", bufs=8))
        opool = ctx.enter_context(tc.tile_pool(name="o", bufs=3))
        wmpool = ctx.enter_context(tc.tile_pool(name="wm", bufs=1))
        pcp = ctx.enter_context(
            tc.tile_pool(name="pc", bufs=2, space=bass.MemorySpace.PSUM))
        pwp = ctx.enter_context(
            tc.tile_pool(name="pw", bufs=5, space=bass.MemorySpace.PSUM))

        # PE warm-up: HAM unthrottles only after ~3.4us of sustained PE
        # activity, and the input DMA ramp takes ~3us to deliver the first
        # real operands.  Burn that dead window with dummy matmuls so the
        # real ones start at full clock.
        warm_sb = wmpool.tile([128, 64], BF16)
        nc.vector.memset(warm_sb[:], 0.0)
        warm_ps = pcp.tile([64, 64], F32, tag="warm", bufs=1)
        for _ in range(NWARM):
            nc.tensor.matmul(warm_ps[:, :], warm_sb[:, :], warm_sb[:, :],
                             start=True, stop=True)

        # One sync-HWDGE queue in exact consumption order: wc, then per
        # m-tile its x slice followed by its W chunks.  PE tracks the stream
        # with ~0.5us lag instead of waiting for one big x load.
        wc_sb = wcpool.tile([128, KCH * ncls_p], F8)
        nc.sync.dma_start(wc_sb[:], wcT[:])
        x_sbs = []
        w_sbs = []
        for m in range(n_mt):
            x_sb = xpool.tile([128, KCH * 128], BF16, tag="x")
            nc.sync.dma_start(
                x_sb[:], xT[:, m * KCH * 128:(m + 1) * KCH * 128])
            x_sbs.append(x_sb)
            row = []
            for (h0, hn) in wchunks(m):
                w_sb = wpool.tile([128, hn * hw], F8, tag="w")
                nc.sync.dma_start(
                    w_sb[:], wwT[m][:, h0 * hw:(h0 + hn) * hw])
                row.append((h0, hn, w_sb))
            w_sbs.append(row)

        for m in range(n_mt):
            def xcol(j, x_sb=x_sbs[m]):
                return x_sb[:, j * 128:(j + 1) * 128]

            # class logits for these 128 tokens
            pc_ps = pcp.tile([128, ncls_p], F32, tag="pc")
            for j in range(KCH):
                nc.tensor.matmul(
                    pc_ps[:, :],
                    xcol(j),
                    wc_sb[:, j * ncls_p:(j + 1) * ncls_p],
                    start=(j == 0), stop=(j == KCH - 1),
                )
            o_sb = opool.tile([128, ocol], BF16)
            nc.scalar.copy(o_sb[:, :NCLS], pc_ps[:, :NCLS])

            # word logits: per half, one M=128 matmul of N=gw covering gs
            # classes; each gs*C-row band keeps its full pair block
            for (h0, hn, w_sb) in w_sbs[m]:
                for hh in range(hn):
                    h = h0 + hh
                    pw_ps = pwp.tile([128, gw], F32, tag="pw")
                    for j in range(KCH):
                        nc.tensor.matmul(
                            pw_ps[:, :],
                            xcol(j),
                            w_sb[:, (hh * KCH + j) * gw:
                                 (hh * KCH + j + 1) * gw],
                            start=(j == 0), stop=(j == KCH - 1),
                        )
                    b0, b1 = h * gs * C, (h + 1) * gs * C
                    nc.vector.tensor_copy(
                        o_sb[b0:b1, NCLS:], pw_ps[b0:b1, :])
                    # store each finished 64-row half so the last store
                    # after the final W chunk is small
                    if n_half >= 2 and b1 in (64, 128):
                        r0 = b1 - 64
                        nc.scalar.dma_start(
                            out[m * 128 + r0:m * 128 + b1, :],
                            o_sb[r0:b1, :])

            if n_half < 2:
                nc.scalar.dma_start(out[m * 128:(m + 1) * 128, :], o_sb[:])

    nc.compile()
    return nc


def _route(cls):
    """Group tokens by class into capacity-padded slots: one slot per class,
    C tokens of capacity.  The (rare) tokens beyond a class's capacity are
    returned as `overflow` and evaluated directly on the host in numpy.

    Returns (C, slots, tok_idx [NCORES, slots*C] int64 token id or -1,
    slot_cls [NCORES, slots] class id per slot, overflow token-id array).
    """
    counts = np.bincount(cls, minlength=NCLS_PAD)
    cands = (16, 32, 64, 128)
    C = cands[-1]
    for c in cands:
        if int(np.maximum(counts - c, 0).sum()) <= 32:
            C = c
            break

    order = np.argsort(cls, kind="stable")
    starts = np.zeros(NCLS_PAD + 1, np.int64)
    starts[1:] = np.cumsum(counts)

    slots = CPC  # one slot per class owned by the core
    tok_idx = np.full((NCORES, slots * C), -1, np.int64)
    slot_cls = np.full((NCORES, slots), -1, np.int64)
    overflow = []
    for k in range(NCORES):
        for s in range(slots):
            c = k * CPC + s
            lo, cnt = int(starts[c]), int(counts[c])
            n = min(C, cnt)
            slot_cls[k, s] = c
            if n > 0:
                tok_idx[k, s * C:s * C + n] = order[lo:lo + n]
            if cnt > C:
                overflow.append(order[lo + C:lo + cnt])
    overflow = (np.concatenate(overflow) if overflow
                else np.zeros((0,), np.int64))
    return C, slots, tok_idx, slot_cls, overflow


def kernel(x, Wc, bc, Ww, bw, cls_idx, _trace=False, _trace_cores=None):
    global LAST_RESULT

    x = np.ascontiguousarray(np.asarray(x, np.float32))
    Wc = np.ascontiguousarray(np.asarray(Wc, np.float32))
    bc = np.asarray(bc, np.float32)
    Ww = np.ascontiguousarray(np.asarray(Ww, np.float32))
    bw = np.asarray(bw, np.float32)
    cls = np.asarray(cls_idx).astype(np.int64).ravel()
    N = cls.shape[0]

    C, slots, tok_idx, slot_cls, overflow = _route(cls)
    npad = slots * C
    n_mt = npad // 128
    per_mt = 128 // C
    gs = 2 if per_mt >= 2 else 1
    ncls_p = 256

    key = (C, slots)
    if key not in _program_cache:
        _program_cache[key] = _build_program(C, slots)
    nc = _program_cache[key]

    # wcT [128, KCH*ncls_p]: wcT[p, j*ncls_p+c] = Wc[c, j*128+p]  (replicated)
    Wc_p = np.concatenate(
        [Wc, np.zeros((ncls_p - NCLS, NHID), np.float32)], 0)
    wcT = np.ascontiguousarray(
        (Wc_p.reshape(ncls_p, KCH, 128).transpose(2, 1, 0)
         .reshape(128, KCH * ncls_p) * WSCALE).astype(_F8_NP))

    Ww_pad = np.zeros((NCLS_PAD, CHUNK, NHID), np.float32)
    Ww_pad[:NCLS] = Ww

    in_maps = []
    for k in range(NCORES):
        # per-slot k-major weights: tmp[s, j, p, w] = Ww[cls_s, w, j*128+p]
        wsel = Ww_pad[np.maximum(slot_cls[k], 0)]
        wsel[slot_cls[k] < 0] = 0.0
        tmp = wsel.reshape(slots, CHUNK, KCH, 128).transpose(0, 2, 3, 1)
        if gs == 2:
            # group = m-tile (per_mt slots); within: pair h, then j, then
            # the two slots' CHUNK columns side by side
            tmp = tmp.reshape(n_mt, per_mt // 2, 2, KCH, 128, CHUNK)
            tmp = tmp.transpose(0, 4, 1, 3, 2, 5)  # [n_mt,128,pair,j,2,CHUNK]
        else:
            tmp = tmp.reshape(n_mt, per_mt, KCH, 128, CHUNK)
            tmp = tmp.transpose(0, 3, 1, 2, 4)     # [n_mt,128,q,j,CHUNK]
        wwT = np.ascontiguousarray(
            (tmp.reshape(n_mt, 128, per_mt * KCH * CHUNK) * WSCALE)
            .astype(_F8_NP))

        ti = tok_idx[k]
        xk = x[np.maximum(ti, 0)]
        xk[ti < 0] = 0.0
        # xT[p, (m*KCH+j)*128 + t] = xk[m*128+t, j*128+p]
        xT = np.ascontiguousarray(
            xk.reshape(n_mt, 128, KCH, 128).transpose(3, 0, 2, 1)
              .reshape(128, n_mt * KCH * 128).astype(_BF16_NP))
        in_maps.append({"xT": xT, "wcT": wcT, "wwT": wwT})

    LAST_RESULT = run_bass_kernel_spmd(
        nc, in_maps, list(range(NCORES)), trace=_trace,
        trace_cores=(_trace_cores if _trace else None))

    out = np.zeros((N, NCOL), np.float32)
    if gs == 2:
        # row r of a core's output holds its pair's full 2*CHUNK block;
        # slot parity selects which CHUNK half is this row's class
        a_row = (np.arange(npad) // C) % 2
    for k in range(NCORES):
        ok = np.asarray(LAST_RESULT.results[k]["out"], np.float32)
        if gs == 2:
            words = np.where((a_row == 0)[:, None],
                             ok[:, NCLS:NCLS + CHUNK],
                             ok[:, NCLS + CHUNK:NCLS + 2 * CHUNK])
            ok = np.concatenate([ok[:, :NCLS], words], 1)
        ok *= (1.0 / WSCALE)
        valid = tok_idx[k] >= 0
        out[tok_idx[k][valid]] = ok[valid]

    if overflow.size:
        # rare capacity-overflow tokens: evaluate directly on the host
        xo = x[overflow]                                   # [no, NHID]
        out[overflow, :NCLS] = xo @ Wc.T
        co = cls[overflow]
        out[overflow, NCLS:] = np.einsum(
            "nkh,nh->nk", Ww[co], xo, optimize=True)

    out[:, :NCLS] += bc
    out[:, NCLS:] += bw[cls]
    return out
